# revision 4
# baseline (speedup 1.0000x reference)
"""CSPN 3x3 per-pixel MAC kernel for Trainium2, 8-core data parallel.

out[b,0,h,w] = sum_{t in 0..8, t!=4} K[b,t,h,w] * xpad[b,h+t//3,w+t%3]
             + K[b,4,t,h,w] * input0[b,0,h,w]

Sharding: batch 16 -> 2 samples per core, pure data parallel.

All tensors are bf16 on device (harness rel-err gate is 2e-2; measured
bf16-chain error is 4.8e-3): halves HBM traffic vs f32 AND engages the
DVE 2x_1p fast path (2 elem/cycle/partition for 2-byte packed
operands), halving compute time too.  Host converts f32->bf16 before
upload and upcasts the bf16 output to f32 after download.

Host-side repack: kern is stored ROW-MAJOR-BY-OUTPUT-ROW as
[SPC, H, 10, W] where record [r] = 9 kernel planes for row r plus the
x0 row -- so a band's entire per-row weight data is ONE dma_start of
p contiguous 24.3KB runs (sequential HBM walk, minimal descriptors).
A device-side [p,9|10,W] walk over the natural [9,H,W] layout was
measured SLOWER than per-plane loads (partition-major descriptor order
jumps 856KB between planes -> HBM scatter).

Layout: partition dim = image rows, bands of 128/128/96; free dim =
width.  Vertical taps come from 3 row-shifted views of the zero-padded
input loaded as ONE overlapping-AP dma_start; horizontal taps are
free-dim offsets.

All 17 elementwise ops (9 mult + 8 add per band) run on DVE: measured
GpSimd co-execution with DVE serializes on this toolchain and hurts,
so the kernel is DVE-only for compute.
"""

import os
import sys

for _p in ("/opt/trn_rl_repo", "/root/.axon_site/_ro/trn_rl_repo"):
    if os.path.isdir(_p) and _p not in sys.path:
        sys.path.append(_p)

import ml_dtypes
import numpy as np

import concourse.bacc as bacc
import concourse.mybir as mybir
from concourse import bass_utils, tile
from concourse.ap import AP

KS = 3
BS, H, W = 16, 352, 1216
NCORES = 8
SPC = BS // NCORES          # samples per core = 2
HP, WP = H + 2, W + 2       # zero-padded dims: 354 x 1218
BF16 = mybir.dt.bfloat16
NP_BF16 = ml_dtypes.bfloat16
MULT = mybir.AluOpType.mult
ADD = mybir.AluOpType.add

ROW_BANDS = [(0, 128), (128, 128), (256, 96)]

BUFS = (2, 4, 4)   # kpool, xpool, apool
NQUEUES = 2        # 1: all DMA on sync(SP); 2: alternate sync/scalar


def _build_nc(loop_reps=1):
    nc = bacc.Bacc(None)
    # [SPC, H, 10, W]: per output row, 9 kernel planes + the x0 row
    kern = nc.dram_tensor("kern", [SPC, H, 10, W], BF16, kind="ExternalInput")
    xpad = nc.dram_tensor("xpad", [SPC, HP, WP], BF16, kind="ExternalInput")
    out = nc.dram_tensor("out", [SPC, H, W], BF16, kind="ExternalOutput")

    xpad_h = xpad[0, 0:1, :].tensor  # underlying handle for raw APs

    qn = [0]

    def q():
        if NQUEUES == 1:
            return nc.sync
        qn[0] += 1
        return nc.scalar if qn[0] % 2 else nc.sync

    with tile.TileContext(nc) as tc:
        with (
            tc.tile_pool(name="kpool", bufs=BUFS[0]) as kpool,
            tc.tile_pool(name="xpool", bufs=BUFS[1]) as xpool,
            tc.tile_pool(name="apool", bufs=BUFS[2]) as apool,
            tc.tile_pool(name="ppool", bufs=1) as ppool,
        ):
            def body():
                for b in range(SPC):
                    for r0, p in ROW_BANDS:
                        xt = xpool.tile([128, 3, WP], BF16, tag="xt")
                        kxt = kpool.tile([128, 10, W], BF16, tag="kxt")

                        # all 3 row-shifted xpad views in one DMA: rows
                        # (r0+i+part) for i=0..2 as an overlapping AP
                        q().dma_start(
                            out=xt[:p, :, :],
                            in_=AP(
                                xpad_h,
                                b * HP * WP + r0 * WP,
                                [[WP, p], [WP, 3], [1, WP]],
                            ),
                        )
                        # one contiguous 24.3KB run per partition
                        q().dma_start(
                            out=kxt[:p, :, :],
                            in_=kern[b, r0 : r0 + p, :, :],
                        )

                        acc = apool.tile([128, W], BF16, tag="acc")
                        prodd = ppool.tile([128, W], BF16, tag="prodd")

                        def src(t):
                            if t == 4:
                                return kxt[:p, 9, :]
                            i, j = t // 3, t % 3
                            return xt[:p, i, j : j + W]

                        for t in range(9):
                            dst = acc[:p, :] if t == 0 else prodd[:p, :]
                            nc.vector.tensor_tensor(
                                out=dst, in0=kxt[:p, t, :],
                                in1=src(t), op=MULT,
                            )
                            if t:
                                nc.vector.tensor_tensor(
                                    out=acc[:p, :],
                                    in0=acc[:p, :],
                                    in1=prodd[:p, :], op=ADD,
                                )
                        q().dma_start(
                            out=out[b, r0 : r0 + p, :], in_=acc[:p, :]
                        )

            if loop_reps == 1:
                body()
            else:
                with tc.For_i(0, loop_reps, 1):
                    body()
    nc.finalize()
    return nc


_NC_CACHE = None


def _get_nc():
    global _NC_CACHE
    if _NC_CACHE is None:
        _NC_CACHE = _build_nc()
    return _NC_CACHE


def _make_in_maps(kernel_arr, input_arr, input0_arr):
    kernel_arr = np.asarray(kernel_arr, dtype=np.float32).astype(NP_BF16)
    inp = np.asarray(input_arr, dtype=np.float32)[:, 0]
    inp0 = np.asarray(input0_arr, dtype=np.float32)[:, 0].astype(NP_BF16)

    # pack: [BS, H, 10, W] = 9 kernel planes + x0, per output row
    kx = np.empty((BS, H, 10, W), dtype=NP_BF16)
    kx[:, :, :9, :] = kernel_arr.transpose(0, 2, 1, 3)
    kx[:, :, 9, :] = inp0

    xp = np.zeros((BS, HP, WP), dtype=NP_BF16)
    xp[:, 1 : H + 1, 1 : W + 1] = inp.astype(NP_BF16)

    in_maps = []
    for c in range(NCORES):
        s = slice(c * SPC, (c + 1) * SPC)
        in_maps.append(
            {
                "kern": np.ascontiguousarray(kx[s]),
                "xpad": np.ascontiguousarray(xp[s]),
            }
        )
    return in_maps


def _run(kernel_arr, input_arr, input0_arr, trace=False):
    in_maps = _make_in_maps(kernel_arr, input_arr, input0_arr)
    nc = _get_nc()
    res = bass_utils.run_bass_kernel_spmd(
        nc, in_maps, list(range(NCORES)), trace=trace
    )
    out = np.concatenate([res.results[c]["out"] for c in range(NCORES)], axis=0)
    out = out.astype(np.float32)
    return np.ascontiguousarray(out.reshape(BS, 1, H, W)), res


def kernel(kernel, input, input0):  # noqa: A002 - names fixed by harness
    out, _ = _run(kernel, input, input0, trace=False)
    return out


# revision 9
# speedup vs baseline: 1.0143x; 1.0143x over previous
"""CSPN 3x3 per-pixel MAC kernel for Trainium2, 8-core data parallel.

out[b,0,h,w] = sum_{t in 0..8, t!=4} K[b,t,h,w] * xpad[b,h+t//3,w+t%3]
             + K[b,4,h,w] * input0[b,0,h,w]

Sharding: batch 16 -> 2 samples per core, pure data parallel.

All tensors are bf16 on device (harness rel-err gate is 2e-2; this
kernel measures 2.7e-3): halves HBM traffic vs f32 AND engages the DVE
2x_1p fast path (2 elem/cycle/partition for 2-byte packed operands).
Host converts f32->bf16 before upload and upcasts the bf16 output to
f32 after download.

Host-side repack: kern is stored ROW-MAJOR-BY-OUTPUT-ROW as
[SPC, H, 10, W] where record [r] = 9 kernel planes for row r plus the
x0 row -- so a band's entire per-row weight data is ONE dma_start of
p contiguous 24.3KB runs (sequential HBM walk, minimal descriptors).
A device-side transposed walk over the natural [9,H,W] layout was
measured ~10% SLOWER end-to-end (partition-major descriptor order
jumps 856KB between planes -> HBM scatter).

Work split across engines (measured: DVE compute and DMA traffic
contend heavily on this silicon -- independent 75us DVE + 75us DMA
streams co-run in 110us -- so spreading compute off DVE is the win):
 - DVE: 9 elementwise products per band into a [p,9,W] bf16 tile.
 - PE:  accumulates the 9 product planes into f32 PSUM via
   identity-weight matmuls (out += I.T @ prod_t), 3 PSUM-bank-sized
   column chunks x 9 taps; f32 accumulation also improves accuracy
   over a bf16 add chain.
 - ACT: copies the f32 PSUM result to a bf16 SBUF tile (DMA cannot
   read PSUM) and issues the x/out DMAs from its HWDGE ring.
 - SP:  issues the big kern DMAs from its HWDGE ring.

GpSimd co-execution was measured 30% slower (serializes against DVE on
this toolchain); partition-base-shifted compute operands and bf16 DVE
writes to PSUM are rejected by the BIR verifier/compiler.

DMA queues are DEDICATED (kern->SP, x/out->ACT): measured ~4us better
than round-robin.  Splitting the band kern load into partition chunks
is much worse (DMA-to-SBUF write throughput scales with destination
partition count): always DMA into all partitions at once.
"""

import os
import sys

for _p in ("/opt/trn_rl_repo", "/root/.axon_site/_ro/trn_rl_repo"):
    if os.path.isdir(_p) and _p not in sys.path:
        sys.path.append(_p)

import ml_dtypes
import numpy as np

import concourse.bacc as bacc
import concourse.mybir as mybir
from concourse import bass_utils, tile
from concourse.ap import AP

KS = 3
BS, H, W = 16, 352, 1216
NCORES = 8
SPC = BS // NCORES          # samples per core = 2
HP, WP = H + 2, W + 2       # zero-padded dims: 354 x 1218
BF16 = mybir.dt.bfloat16
F32 = mybir.dt.float32
NP_BF16 = ml_dtypes.bfloat16
MULT = mybir.AluOpType.mult

ROW_BANDS = [(0, 128), (128, 128), (256, 96)]
# PSUM-bank-sized (512 f32) column chunks of W for matmul accumulation
CHUNKS = [(0, 512), (512, 512), (1024, 192)]


def _build_nc(loop_reps=1):
    nc = bacc.Bacc(None)
    # [SPC, H, 10, W]: per output row, 9 kernel planes + the x0 row
    kern = nc.dram_tensor("kern", [SPC, H, 10, W], BF16, kind="ExternalInput")
    xpad = nc.dram_tensor("xpad", [SPC, HP, WP], BF16, kind="ExternalInput")
    ident = nc.dram_tensor("ident", [128, 128], BF16, kind="ExternalInput")
    out = nc.dram_tensor("out", [SPC, H, W], BF16, kind="ExternalOutput")

    xpad_h = xpad[0, 0:1, :].tensor  # underlying handle for raw APs

    with tile.TileContext(nc) as tc:
        with (
            tc.tile_pool(name="ipool", bufs=1) as ipool,
            tc.tile_pool(name="kpool", bufs=2) as kpool,
            tc.tile_pool(name="xpool", bufs=4) as xpool,
            tc.tile_pool(name="prpool", bufs=2) as prpool,
            tc.tile_pool(name="pspool", bufs=2, space="PSUM") as pspool,
            tc.tile_pool(name="opool", bufs=4) as opool,
        ):
            it = ipool.tile([128, 128], BF16, tag="ident")
            nc.sync.dma_start(out=it[:, :], in_=ident[:, :])

            def body():
                for b in range(SPC):
                    for r0, p in ROW_BANDS:
                        kxt = kpool.tile([128, 10, W], BF16, tag="kxt")
                        xt = xpool.tile([128, 3, WP], BF16, tag="xt")
                        # all 3 row-shifted xpad views in one DMA
                        # (overlapping AP, rows r0+i+part for i=0..2)
                        nc.scalar.dma_start(
                            out=xt[:p, :, :],
                            in_=AP(
                                xpad_h,
                                b * HP * WP + r0 * WP,
                                [[WP, p], [WP, 3], [1, WP]],
                            ),
                        )
                        # one contiguous 24.3KB run per partition
                        nc.sync.dma_start(
                            out=kxt[:p, :, :],
                            in_=kern[b, r0 : r0 + p, :, :],
                        )

                        prod = prpool.tile([128, 9, W], BF16, tag="prod")
                        ps = pspool.tile([128, W], F32, tag="ps")
                        ot = opool.tile([128, W], BF16, tag="ot")

                        def src(t):
                            if t == 4:
                                return kxt[:p, 9, :]
                            i, j = t // 3, t % 3
                            return xt[:p, i, j : j + W]

                        for t in range(9):
                            nc.vector.tensor_tensor(
                                out=prod[:p, t, :], in0=kxt[:p, t, :],
                                in1=src(t), op=MULT,
                            )
                            for w0, wc in CHUNKS:
                                nc.tensor.matmul(
                                    out=ps[:p, w0 : w0 + wc],
                                    lhsT=it[:p, :p],
                                    rhs=prod[:p, t, w0 : w0 + wc],
                                    start=(t == 0), stop=(t == 8),
                                )
                        nc.scalar.copy(out=ot[:p, :], in_=ps[:p, :])
                        nc.scalar.dma_start(
                            out=out[b, r0 : r0 + p, :], in_=ot[:p, :]
                        )

            if loop_reps == 1:
                body()
            else:
                with tc.For_i(0, loop_reps, 1):
                    body()
    nc.finalize()
    return nc


_NC_CACHE = None


def _get_nc():
    global _NC_CACHE
    if _NC_CACHE is None:
        _NC_CACHE = _build_nc()
    return _NC_CACHE


def _make_in_maps(kernel_arr, input_arr, input0_arr):
    kernel_arr = np.asarray(kernel_arr, dtype=np.float32).astype(NP_BF16)
    inp = np.asarray(input_arr, dtype=np.float32)[:, 0]
    inp0 = np.asarray(input0_arr, dtype=np.float32)[:, 0].astype(NP_BF16)

    # pack: [BS, H, 10, W] = 9 kernel planes + x0, per output row
    kx = np.empty((BS, H, 10, W), dtype=NP_BF16)
    kx[:, :, :9, :] = kernel_arr.transpose(0, 2, 1, 3)
    kx[:, :, 9, :] = inp0

    xp = np.zeros((BS, HP, WP), dtype=NP_BF16)
    xp[:, 1 : H + 1, 1 : W + 1] = inp.astype(NP_BF16)

    ident = np.eye(128, dtype=NP_BF16)

    in_maps = []
    for c in range(NCORES):
        s = slice(c * SPC, (c + 1) * SPC)
        in_maps.append(
            {
                "kern": np.ascontiguousarray(kx[s]),
                "xpad": np.ascontiguousarray(xp[s]),
                "ident": ident,
            }
        )
    return in_maps


def _run(kernel_arr, input_arr, input0_arr, trace=False):
    in_maps = _make_in_maps(kernel_arr, input_arr, input0_arr)
    nc = _get_nc()
    res = bass_utils.run_bass_kernel_spmd(
        nc, in_maps, list(range(NCORES)), trace=trace
    )
    out = np.concatenate([res.results[c]["out"] for c in range(NCORES)], axis=0)
    out = out.astype(np.float32)
    return np.ascontiguousarray(out.reshape(BS, 1, H, W)), res


def kernel(kernel, input, input0):  # noqa: A002 - names fixed by harness
    out, _ = _run(kernel, input, input0, trace=False)
    return out


# revision 10
# speedup vs baseline: 1.1096x; 1.0940x over previous
"""CSPN 3x3 per-pixel MAC kernel for Trainium2, 8-core data parallel.

out[b,0,h,w] = sum_{t in 0..8, t!=4} K[b,t,h,w] * xpad[b,h+t//3,w+t%3]
             + K[b,4,h,w] * input0[b,0,h,w]

Sharding: batch 16 -> 2 samples per core, pure data parallel.

All tensors are bf16 on device (harness rel-err gate is 2e-2; this
kernel measures 4.8e-3): halves HBM traffic vs f32 AND engages the DVE
2x_1p fast path (2 elem/cycle/partition for 2-byte packed operands),
halving compute time too.  Host converts f32->bf16 before upload and
upcasts the bf16 output to f32 after download.

Host-side repack: kern is stored ROW-MAJOR-BY-OUTPUT-ROW as
[SPC, H, 10, W] where record [r] = 9 kernel planes for row r plus the
x0 row -- so a band's entire per-row weight data is ONE dma_start of
p contiguous 24.3KB runs (sequential HBM walk, minimal descriptors).
A device-side transposed walk over the natural [9,H,W] layout was
measured ~10% slower end-to-end (partition-major descriptor order
jumps 856KB between planes -> HBM scatter).

Layout: partition dim = image rows, bands of 128/128/96; free dim =
width.  Vertical taps come from 3 row-shifted views of the zero-padded
input loaded as ONE overlapping-AP dma_start; horizontal taps are
free-dim offsets.

DMA queues are DEDICATED (kern->SP ring, x/out->ACT ring): measured
~4us better than round-robin.  Splitting the band kern load into
partition chunks is much worse (DMA-to-SBUF write throughput scales
with destination partition count): always DMA into all partitions at
once.

All 17 elementwise ops (9 mult + 8 add per band) run on DVE.
Negative results measured on this silicon/toolchain, do not retry:
 - GpSimd co-execution (width-split): 30% slower, serializes vs DVE.
 - PE identity-matmul accumulation of DVE products into PSUM (DVE
   busy 75->37us): total unchanged (~112us) -- the wall is DMA-vs-
   compute SBUF contention (DMA runs ~320GB/s alone, ~218GB/s with
   any engine co-running), not engine busy time.
 - Partition-base-shifted compute operands: rejected by BIR verifier.
 - bf16 DVE writes to PSUM: rejected by compiler (matmul/memset only).
 - SBUF->SBUF partition-shift copies to single-read x: 134us (SBUF
   DMA bytes are the contended resource, HBM re-read is cheaper).
 - SWDGE (Pool-engine) DMA for the kern load: does not execute
   correctly inside hardware loops here.
"""

import os
import sys

for _p in ("/opt/trn_rl_repo", "/root/.axon_site/_ro/trn_rl_repo"):
    if os.path.isdir(_p) and _p not in sys.path:
        sys.path.append(_p)

import ml_dtypes
import numpy as np

import concourse.bacc as bacc
import concourse.mybir as mybir
from concourse import bass_utils, tile
from concourse.ap import AP

KS = 3
BS, H, W = 16, 352, 1216
NCORES = 8
SPC = BS // NCORES          # samples per core = 2
HP, WP = H + 2, W + 2       # zero-padded dims: 354 x 1218
BF16 = mybir.dt.bfloat16
NP_BF16 = ml_dtypes.bfloat16
MULT = mybir.AluOpType.mult
ADD = mybir.AluOpType.add

ROW_BANDS = [(0, 128), (128, 128), (256, 96)]

BUFS = (2, 4, 4)   # kpool, xpool, apool


def _build_nc(loop_reps=1):
    nc = bacc.Bacc(None)
    # [SPC, H, 10, W]: per output row, 9 kernel planes + the x0 row
    kern = nc.dram_tensor("kern", [SPC, H, 10, W], BF16, kind="ExternalInput")
    xpad = nc.dram_tensor("xpad", [SPC, HP, WP], BF16, kind="ExternalInput")
    out = nc.dram_tensor("out", [SPC, H, W], BF16, kind="ExternalOutput")

    xpad_h = xpad[0, 0:1, :].tensor  # underlying handle for raw APs

    with tile.TileContext(nc) as tc:
        with (
            tc.tile_pool(name="kpool", bufs=BUFS[0]) as kpool,
            tc.tile_pool(name="xpool", bufs=BUFS[1]) as xpool,
            tc.tile_pool(name="apool", bufs=BUFS[2]) as apool,
            tc.tile_pool(name="ppool", bufs=1) as ppool,
        ):
            def body():
                for b in range(SPC):
                    for r0, p in ROW_BANDS:
                        kxt = kpool.tile([128, 10, W], BF16, tag="kxt")
                        xt = xpool.tile([128, 3, WP], BF16, tag="xt")
                        # all 3 row-shifted xpad views in one DMA: rows
                        # (r0+i+part) for i=0..2 as an overlapping AP
                        nc.scalar.dma_start(
                            out=xt[:p, :, :],
                            in_=AP(
                                xpad_h,
                                b * HP * WP + r0 * WP,
                                [[WP, p], [WP, 3], [1, WP]],
                            ),
                        )
                        # one contiguous 24.3KB run per partition
                        nc.sync.dma_start(
                            out=kxt[:p, :, :],
                            in_=kern[b, r0 : r0 + p, :, :],
                        )

                        acc = apool.tile([128, W], BF16, tag="acc")
                        prodd = ppool.tile([128, W], BF16, tag="prodd")

                        def src(t):
                            if t == 4:
                                return kxt[:p, 9, :]
                            i, j = t // 3, t % 3
                            return xt[:p, i, j : j + W]

                        for t in range(9):
                            dst = acc[:p, :] if t == 0 else prodd[:p, :]
                            nc.vector.tensor_tensor(
                                out=dst, in0=kxt[:p, t, :],
                                in1=src(t), op=MULT,
                            )
                            if t:
                                nc.vector.tensor_tensor(
                                    out=acc[:p, :],
                                    in0=acc[:p, :],
                                    in1=prodd[:p, :], op=ADD,
                                )
                        nc.scalar.dma_start(
                            out=out[b, r0 : r0 + p, :], in_=acc[:p, :]
                        )

            if loop_reps == 1:
                body()
            else:
                with tc.For_i(0, loop_reps, 1):
                    body()
    nc.finalize()
    return nc


_NC_CACHE = None


def _get_nc():
    global _NC_CACHE
    if _NC_CACHE is None:
        _NC_CACHE = _build_nc()
    return _NC_CACHE


def _make_in_maps(kernel_arr, input_arr, input0_arr):
    kernel_arr = np.asarray(kernel_arr, dtype=np.float32).astype(NP_BF16)
    inp = np.asarray(input_arr, dtype=np.float32)[:, 0]
    inp0 = np.asarray(input0_arr, dtype=np.float32)[:, 0].astype(NP_BF16)

    # pack: [BS, H, 10, W] = 9 kernel planes + x0, per output row
    kx = np.empty((BS, H, 10, W), dtype=NP_BF16)
    kx[:, :, :9, :] = kernel_arr.transpose(0, 2, 1, 3)
    kx[:, :, 9, :] = inp0

    xp = np.zeros((BS, HP, WP), dtype=NP_BF16)
    xp[:, 1 : H + 1, 1 : W + 1] = inp.astype(NP_BF16)

    in_maps = []
    for c in range(NCORES):
        s = slice(c * SPC, (c + 1) * SPC)
        in_maps.append(
            {
                "kern": np.ascontiguousarray(kx[s]),
                "xpad": np.ascontiguousarray(xp[s]),
            }
        )
    return in_maps


def _run(kernel_arr, input_arr, input0_arr, trace=False):
    in_maps = _make_in_maps(kernel_arr, input_arr, input0_arr)
    nc = _get_nc()
    res = bass_utils.run_bass_kernel_spmd(
        nc, in_maps, list(range(NCORES)), trace=trace
    )
    out = np.concatenate([res.results[c]["out"] for c in range(NCORES)], axis=0)
    out = out.astype(np.float32)
    return np.ascontiguousarray(out.reshape(BS, 1, H, W)), res


def kernel(kernel, input, input0):  # noqa: A002 - names fixed by harness
    out, _ = _run(kernel, input, input0, trace=False)
    return out
